# revision 39
# baseline (speedup 1.0000x reference)
"""Trainium2 Bass kernel for nn_BrainGeneratorModel (bias-field corrupt + per-sample
separable Gaussian blur + label LUT remap), 8-core data/spatial parallel.

Sharding: 8 cores = (sample b in 0..3) x (H-half in 0..1). Each core processes a
[D=192, H=96(+12 halo), W=192] subvolume of one sample plus its label slice.

Per-core pipeline:
  Image (PE/ACT/DVE, zero transposes via stationary-swap matmuls):
    A) stream d-batches: bias matmul (K=4) -> exp (ACT) -> x*expb (DVE tt, bf16)
       -> h-blur with xb as stationary (out partitions = w) -> zq [w,(h',d)] SBUF
    B) stream h'-batches: w-blur with zq-slices as stationary (out partitions = d)
       -> zdd [d,(hl,w')] -> d-blur (normal form) -> PSUM -> img DRAM (f32, direct)
  Labels (DVE and POOL in parallel):
    DVE half: 7 bitplane extractions (Mj >> label) & 1 (reverse0 tensor_scalar)
       + Horner accumulate -> dest value directly.
    POOL half: pairs c = l0 + 32*l1 (host-built) -> gpsimd ap_gather against a
       1024-entry packed LUT (T[l0] + 128*T[l1], int32) -> host unpacks.
"""

import sys

for _p in ("/opt/trn_rl_repo",):
    if _p not in sys.path:
        sys.path.insert(0, _p)

import numpy as np
import ml_dtypes

import concourse.bass as bass
import concourse.mybir as mybir
import concourse.bacc as bacc
import concourse.tile as tile
from concourse.bass_utils import run_bass_kernel_spmd

F32 = mybir.dt.float32
BF16 = mybir.dt.bfloat16
I16 = mybir.dt.int16
I32 = mybir.dt.int32

B, C, D, H, W = 4, 1, 192, 192, 192
SMALL = 4
BIAS_STD = 0.7
MAX_SIGMA = 3.0
TRUNCATE = 4.0
K = 2 * int(TRUNCATE * MAX_SIGMA) + 1  # 25
P = K // 2  # 12
N_LABELS = 32
TABLE = 128

HC = 96            # interior H rows per core
HS = 120           # slab rows = HC + 2*P
DB = 8             # d-batch size (stage A)
NB_A = D // DB     # 24
HB = 8             # h'-batch size (stage B)
NB_B = HC // HB    # 12

FLAB = D * HC * W // 128       # 27648 label cols per partition
CD = 12096                     # DVE-share cols (12 chunks)
CP = FLAB - CD                 # pool share
NPAIR = CP // 2                # 6912 pairs per partition
LCH = 1008                     # DVE label chunk cols
NLC = CD // LCH                # 12 chunks
SC = 144                       # pool idx cols per ap_gather call
NGC = NPAIR // SC              # 54 calls
GF = 16 * SC                   # 2304 gather out free size

_CACHE = {}


def _lin_weights(n_in, n_out):
    pos = np.linspace(0.0, n_in - 1.0, n_out, dtype=np.float64)
    i0 = np.clip(np.floor(pos).astype(np.int64), 0, n_in - 2)
    f = pos - i0
    Wm = np.zeros((n_out, n_in), np.float64)
    r = np.arange(n_out)
    np.add.at(Wm, (r, i0), 1.0 - f)
    np.add.at(Wm, (r, i0 + 1), f)
    return Wm


def _gauss_kernels(sigma3):
    ar = np.arange(K, dtype=np.float64) - K // 2
    out = np.zeros((3, K), np.float64)
    for i, sg in enumerate(sigma3):
        s = max(float(sg), 1e-3)
        g = np.exp(-0.5 * ar * ar / (s * s))
        g = g / g.sum()
        if float(sg) >= 0.01:
            out[i] = g
        else:
            out[i, K // 2] = 1.0
    return out


def _edge_folded_toeplitz(g, n):
    """[n, n] M with out[dst] = sum_src M[src, dst] * x[src], replicate pad."""
    M = np.zeros((n, n), np.float64)
    for j in range(n):
        for t in range(K):
            src = min(max(j + t - P, 0), n - 1)
            M[src, j] += g[t]
    return M


def _slab_toeplitz(g):
    """[HS, HC]: slab rows (pre-clipped by host) -> interior outputs."""
    M = np.zeros((HS, HC), np.float64)
    for j in range(HC):
        for t in range(K):
            M[j + t, j] += g[t]
    return M


def _build_program(masks):
    """masks: 7 int32 bitplane masks of the dest LUT (baked immediates)."""
    nc = bacc.Bacc("TRN2", target_bir_lowering=False, debug=False)

    # ---- external inputs (per core) ----
    xs_h = nc.dram_tensor("xs", [HS, D * W], BF16, kind="ExternalInput")
    c_h = nc.dram_tensor("cydw", [4, D * W], BF16, kind="ExternalInput")
    wht_h = nc.dram_tensor("wht", [4, HS], BF16, kind="ExternalInput")
    gh_h = nc.dram_tensor("gh", [HS, HC], BF16, kind="ExternalInput")
    gw_h = nc.dram_tensor("gw", [W, W], BF16, kind="ExternalInput")
    gd_h = nc.dram_tensor("gd", [D, D], BF16, kind="ExternalInput")
    labd_h = nc.dram_tensor("labd", [128, CD], I32, kind="ExternalInput")
    cpair_h = nc.dram_tensor("cpair", [128, NPAIR], I16, kind="ExternalInput")
    tab2_h = nc.dram_tensor("tab2", [128, 1024], I32, kind="ExternalInput")

    # ---- external outputs ----
    img_h = nc.dram_tensor("img", [D, HC, W], BF16, kind="ExternalOutput")
    labo_h = nc.dram_tensor("labo", [128, CD], I32, kind="ExternalOutput")
    gout_h = nc.dram_tensor("gout", [8, NGC * GF], I32, kind="ExternalOutput")

    with tile.TileContext(nc) as tc:
        with (
            tc.tile_pool(name="consts", bufs=1) as cst,
            tc.tile_pool(name="sxp", bufs=2) as sxp,
            tc.tile_pool(name="cbp", bufs=3) as cbp,
            tc.tile_pool(name="ebp", bufs=3) as ebp,
            tc.tile_pool(name="xbp", bufs=2) as xbp,
            tc.tile_pool(name="zqp", bufs=1) as zqp,
            tc.tile_pool(name="zdp", bufs=2) as zdp,
            tc.tile_pool(name="zip", bufs=2) as zip_,
            tc.tile_pool(name="l32p", bufs=3) as l32p,
            tc.tile_pool(name="ltp", bufs=2) as ltp,
            tc.tile_pool(name="tp", bufs=2) as tp,
            tc.tile_pool(name="accp", bufs=2) as accp,
            tc.tile_pool(name="gp", bufs=2) as gp,  # lag covered by 2 bufs
            tc.tile_pool(name="psA", bufs=3, space="PSUM") as pspA,
            tc.tile_pool(name="ps", bufs=5, space="PSUM") as psp,
        ):
            # ---- constants / persistent tiles ----
            ght = cst.tile([HS, HC], BF16)
            nc.sync.dma_start(ght[:], gh_h.ap())
            gwa = cst.tile([128, W], BF16)
            nc.sync.dma_start(gwa[:], gw_h.ap()[0:128, :])
            gwb = cst.tile([64, W], BF16)
            nc.sync.dma_start(gwb[:], gw_h.ap()[128:192, :])
            gda = cst.tile([128, D], BF16)
            nc.sync.dma_start(gda[:], gd_h.ap()[0:128, :])
            gdb = cst.tile([64, D], BF16)
            nc.sync.dma_start(gdb[:], gd_h.ap()[128:192, :])
            whtt = cst.tile([4, HS], BF16)
            nc.sync.dma_start(whtt[:], wht_h.ap())

            tab2 = cst.tile([128, 1024], I32)
            cpair = cst.tile([128, NPAIR], I16)
            lab_consts_loaded = [False]

            def load_label_consts():
                if not lab_consts_loaded[0]:
                    lab_consts_loaded[0] = True
                    nc.sync.dma_start(tab2[:], tab2_h.ap())
                    nc.sync.dma_start(cpair[:], cpair_h.ap())

            # zq: persistent h/w-blurred volume, [w, (h', d)] bf16
            zqa = zqp.tile([128, HC * D], BF16, tag="zqa")
            zqb = zqp.tile([64, HC * D], BF16, tag="zqb")

            # ================= labels: thunk generators =================
            # DVE bitplane half + POOL ap_gather half, drained into the image
            # loops so every engine queue interleaves label and image work.
            eng = nc.vector

            def _chunk_load(ch):
                sl = slice(ch * LCH, (ch + 1) * LCH)
                l32 = l32p.tile([128, LCH], I32, tag="l32")
                return l32, (lambda: nc.sync.dma_start(
                    l32[:], labd_h.ap()[:, sl]))

            def _one_chunk(ch, l32):
                """Yield compute thunks for one chunk's chain."""
                sl = slice(ch * LCH, (ch + 1) * LCH)
                acc = accp.tile([128, LCH], I32, tag="acc")
                t = tp.tile([128, LCH], I32, tag="t")
                for j in range(6, -1, -1):
                    dst = acc if j == 6 else t

                    def ext(l32=l32, dst=dst, j=j):
                        eng.add_instruction(
                            mybir.InstTensorScalarPtr(
                                name=f"I-{nc.next_id()}",
                                op0=mybir.AluOpType.logical_shift_right,
                                reverse0=True,
                                op1=mybir.AluOpType.bitwise_and,
                                ins=[
                                    eng.lower_ap(l32[:]),
                                    eng.lower_ap_or_imm(int(masks[j]),
                                                        imm_dtype=I32),
                                    eng.lower_ap_or_imm(1, imm_dtype=I32),
                                ],
                                outs=[eng.lower_ap(dst[:])],
                            )
                        )
                    yield ext
                    if j != 6:
                        yield lambda acc=acc, t=t: \
                            nc.vector.scalar_tensor_tensor(
                                acc[:], acc[:], 2, t[:],
                                mybir.AluOpType.mult, mybir.AluOpType.add)
                yield ("defer", lambda acc=acc, sl=sl: nc.sync.dma_start(
                    labo_h.ap()[:, sl], acc[:]))

            def dve_label_thunks():
                # two chunk chains in lockstep to hide per-op latency; loads
                # prefetched one pair ahead; output DMAs deferred one step so
                # they never head-block the SP queue.
                loads = [_chunk_load(ch) for ch in range(2)]
                for th in (loads[0][1], loads[1][1]):
                    th()
                deferred = []
                for cp2 in range((NLC + 1) // 2):
                    # prefetch next pair's loads now (tiles: bufs >= 3)
                    if 2 * cp2 + 2 < NLC:
                        l32n, thn = _chunk_load(2 * cp2 + 2)
                        thn()
                        loads.append((l32n, None))
                    if 2 * cp2 + 3 < NLC:
                        l32n, thn = _chunk_load(2 * cp2 + 3)
                        loads.append((l32n, thn))
                    ga = _one_chunk(2 * cp2, loads[2 * cp2][0])
                    gb = (_one_chunk(2 * cp2 + 1, loads[2 * cp2 + 1][0])
                          if 2 * cp2 + 1 < NLC else iter(()))
                    half_done = False
                    while True:
                        a = next(ga, None)
                        b = next(gb, None)
                        if a is None and b is None:
                            break
                        if not half_done and deferred:
                            # flush prior pair's output DMAs mid-pair
                            for d in deferred:
                                d()
                            deferred = []
                            half_done = True
                        for item in (a, b):
                            if item is None:
                                continue
                            if isinstance(item, tuple) and item[0] == "defer":
                                deferred.append(item[1])
                            else:
                                yield item
                    # late-emitted load of the pair after next
                    if 2 * cp2 + 3 < NLC and loads[2 * cp2 + 3][1] is not None:
                        loads[2 * cp2 + 3][1]()
                        loads[2 * cp2 + 3] = (loads[2 * cp2 + 3][0], None)
                for d in deferred:
                    d()

            def pool_label_thunks():
                pending = []

                def flush():
                    while pending:
                        pending.pop(0)()

                for cp2 in range(NGC // 2):
                    def call(cp2=cp2):
                        g = gp.tile([128, 2 * GF], I32, tag="g")
                        for k in range(2):
                            ca = 2 * cp2 + k
                            nc.gpsimd.ap_gather(
                                g[:, k * GF:(k + 1) * GF], tab2[:],
                                cpair[:, ca * SC:(ca + 1) * SC],
                                128, 1024, 1, GF)
                        flush()
                        pending.append(lambda g=g, cp2=cp2: nc.sync.dma_start(
                            gout_h.ap()[:, 2 * cp2 * GF:(2 * cp2 + 2) * GF],
                            g[0:128:16, :],
                        ))
                    yield call
                yield flush

            dve_it = dve_label_thunks()
            pool_it = pool_label_thunks()
            n_dve_thunks = NLC * 15
            dve_drained = 0.0
            pool_drained = 0.0
            # cumulative pacing weights: stage-A slots lighter (DVE busy with
            # xb mults there), stage-B slots heavier
            WA, WB = 1.0, 5.0
            wtot = NB_A * WA + NB_B * WB

            def drain(slot):
                nonlocal dve_drained, pool_drained
                if slot >= 1:
                    load_label_consts()
                w = ((slot + 1) * WA if slot < NB_A
                     else NB_A * WA + (slot + 1 - NB_A) * WB)
                dve_target = n_dve_thunks * w / wtot
                while dve_drained < dve_target:
                    th = next(dve_it, None)
                    if th is None:
                        break
                    th()
                    dve_drained += 1
                pool_target = (NGC // 2 + 1) * (slot + 1) / (NB_A + NB_B)
                while pool_drained < pool_target:
                    th = next(pool_it, None)
                    if th is None:
                        break
                    th()
                    pool_drained += 1

            # ================= image stage A =================
            # xb = x * exp(bias); h-blur via stationary-swap -> zq [w, (h', d)]
            for ib in range(NB_A):
                    d0 = ib * DB
                    sx = sxp.tile([HS, DB * W], BF16)
                    nc.sync.dma_start(sx[:], xs_h.ap()[:, ib * DB * W:
                                                       (ib + 1) * DB * W])
                    cb = cbp.tile([4, DB * W], BF16)
                    nc.sync.dma_start(cb[:], c_h.ap()[:, ib * DB * W:
                                                      (ib + 1) * DB * W])
                    xb = xbp.tile([HS, DB * W], BF16)
                    for q in range(3):
                        qsl = slice(q * 512, (q + 1) * 512)
                        psb = pspA.tile([HS, 512], F32, tag="psb")
                        nc.tensor.matmul(psb[:], whtt[:], cb[:, qsl],
                                         start=True, stop=True)
                        eb = ebp.tile([HS, 512], BF16, tag="eb")
                        nc.scalar.activation(eb[:], psb[:],
                                             mybir.ActivationFunctionType.Exp)
                        nc.vector.tensor_tensor(xb[:, qsl], sx[:, qsl], eb[:],
                                                mybir.AluOpType.mult)
                    # h-blur: stationary = xb slice [h, w-block], rhs = ght
                    # psum [w-block, (dsub 4, h' 96)]
                    for g4 in range(2):
                        pha = psp.tile([128, 4 * HC], F32, tag="ps")
                        phb = psp.tile([64, 4 * HC], F32, tag="ps")
                        for t4 in range(4):
                            dl = g4 * 4 + t4
                            csl = slice(t4 * HC, (t4 + 1) * HC)
                            nc.tensor.matmul(
                                pha[:, csl],
                                xb[:, dl * W: dl * W + 128], ght[:],
                                start=True, stop=True)
                            nc.tensor.matmul(
                                phb[:, csl],
                                xb[:, dl * W + 128: dl * W + 192], ght[:],
                                start=True, stop=True)
                        dq = d0 + 4 * g4
                        nc.scalar.copy(
                            zqa[:].rearrange("p (h d) -> p d h", d=D)
                               [:, dq:dq + 4, :],
                            pha[:].rearrange("p (t h) -> p t h", t=4),
                        )
                        nc.scalar.copy(
                            zqb[:].rearrange("p (h d) -> p d h", d=D)
                               [:, dq:dq + 4, :],
                            phb[:].rearrange("p (t h) -> p t h", t=4),
                        )
                    drain(ib)

            # ================= image stage B =================
            # w-blur: stationary = zq slice [w, d-block] per h',
            #         rhs = gw [w, w'] -> psum [d-block, w'] -> zdd
            # d-blur: lhsT = gd [d, d'-block], rhs = zdd -> psum -> img DRAM
            img_pending = []
            for jb in range(NB_B):
                h0 = jb * HB
                zda = zdp.tile([128, HB * W], BF16, tag="zda")
                zdb = zdp.tile([64, HB * W], BF16, tag="zdb")
                for hp in range(HB // 2):
                    pwa = psp.tile([128, 2 * W], F32, tag="ps")
                    pwb = psp.tile([64, 2 * W], F32, tag="ps")
                    for t2 in range(2):
                        hl = hp * 2 + t2
                        zoff = (h0 + hl) * D
                        csl = slice(t2 * W, (t2 + 1) * W)
                        nc.tensor.matmul(pwa[:, csl],
                                         zqa[:, zoff: zoff + 128], gwa[:],
                                         start=True, stop=False)
                        nc.tensor.matmul(pwa[:, csl],
                                         zqb[:, zoff: zoff + 128], gwb[:],
                                         start=False, stop=True)
                        nc.tensor.matmul(pwb[:, csl],
                                         zqa[:, zoff + 128: zoff + 192], gwa[:],
                                         start=True, stop=False)
                        nc.tensor.matmul(pwb[:, csl],
                                         zqb[:, zoff + 128: zoff + 192], gwb[:],
                                         start=False, stop=True)
                    nc.scalar.copy(
                        zda[:, hp * 2 * W:(hp + 1) * 2 * W], pwa[:])
                    nc.scalar.copy(
                        zdb[:, hp * 2 * W:(hp + 1) * 2 * W], pwb[:])

                # flush previous group's img DMAs (now certainly ready)
                for th in img_pending:
                    th()
                img_pending.clear()
                zia = zip_.tile([128, HB * W], BF16, tag="zia")
                zib = zip_.tile([64, HB * W], BF16, tag="zib")
                for q in range(HB * W // 512):
                    qsl = slice(q * 512, (q + 1) * 512)
                    pia = psp.tile([128, 512], F32, tag="ps")
                    pib = psp.tile([64, 512], F32, tag="ps")
                    nc.tensor.matmul(pia[:], gda[:, 0:128], zda[:, qsl],
                                     start=True, stop=False)
                    nc.tensor.matmul(pia[:], gdb[:, 0:128], zdb[:, qsl],
                                     start=False, stop=True)
                    nc.tensor.matmul(pib[:], gda[:, 128:192], zda[:, qsl],
                                     start=True, stop=False)
                    nc.tensor.matmul(pib[:], gdb[:, 128:192], zdb[:, qsl],
                                     start=False, stop=True)
                    nc.scalar.copy(zia[:, qsl], pia[:])
                    nc.scalar.copy(zib[:, qsl], pib[:])
                # img [d', h, w]: rows h0..h0+8 contiguous per d'
                img_pending.append(lambda zia=zia, h0=h0: nc.sync.dma_start(
                    bass.AP(img_h, h0 * W, [[HC * W, 128], [1, HB * W]]),
                    zia[:],
                ))
                img_pending.append(lambda zib=zib, h0=h0: nc.sync.dma_start(
                    bass.AP(img_h, 128 * HC * W + h0 * W,
                            [[HC * W, 64], [1, HB * W]]),
                    zib[:],
                ))
                drain(NB_A + jb)

            for th in img_pending:
                th()
            img_pending.clear()
            for th in dve_it:
                th()
            for th in pool_it:
                th()
    nc.compile()
    return nc


def _host_prep(x, small_bias, sigma01, labels, source_values, dest_values):
    Wd = _lin_weights(SMALL, D)
    Whm = _lin_weights(SMALL, H)
    Wwm = _lin_weights(SMALL, W)

    mapping = np.zeros(TABLE, np.int64)
    mapping[np.asarray(source_values, np.int64)] = np.asarray(dest_values, np.int64)
    T = mapping[:N_LABELS]

    c_all = np.arange(1024)
    tab2 = (T[c_all % 32] + 128 * T[c_all // 32]).astype(np.int32)
    tab2_rep = np.broadcast_to(tab2, (128, 1024)).copy()

    in_maps = []
    for c in range(8):
        b, half = c // 2, c % 2
        h0 = half * HC
        hidx = np.clip(np.arange(h0 - P, h0 + HC + P), 0, H - 1)

        # x slab, h-major bf16: [HS, D, W]
        xs = np.asarray(x[b, 0], np.float32)[:, hidx, :].transpose(1, 0, 2)
        xs = np.ascontiguousarray(xs, dtype=ml_dtypes.bfloat16).reshape(HS, D * W)

        sm = np.asarray(small_bias[b, 0], np.float64) * BIAS_STD
        Cydw = np.einsum("xyz,dx,wz->ydw", sm, Wd, Wwm).reshape(4, D * W)
        WhT = np.ascontiguousarray(Whm[hidx, :].T)

        g3 = _gauss_kernels(np.asarray(sigma01[b], np.float64) * MAX_SIGMA)
        Gh = _slab_toeplitz(g3[1])
        Gw = _edge_folded_toeplitz(g3[2], W)
        Gd = _edge_folded_toeplitz(g3[0], D)

        lab = np.asarray(labels[b, 0][:, h0:h0 + HC, :], np.int32).reshape(128, FLAB)
        labd = np.ascontiguousarray(lab[:, :CD])
        lp = lab[:, CD:]
        cpair = (lp[:, NPAIR:] * 32 + lp[:, :NPAIR]).astype(np.int16)

        in_maps.append({
            "xs": xs,
            "cydw": Cydw.astype(ml_dtypes.bfloat16),
            "wht": WhT.astype(ml_dtypes.bfloat16),
            "gh": Gh.astype(ml_dtypes.bfloat16),
            "gw": Gw.astype(ml_dtypes.bfloat16),
            "gd": Gd.astype(ml_dtypes.bfloat16),
            "labd": labd,
            "cpair": np.ascontiguousarray(cpair),
            "tab2": tab2_rep,
        })
    return in_maps, T


def kernel(x, small_bias, sigma01, labels, source_values, dest_values):
    in_maps, T = _host_prep(x, small_bias, sigma01, labels,
                            source_values, dest_values)

    # bitplane masks of the LUT (program immediates -> cache key)
    M = np.zeros(7, dtype=np.uint32)
    for j in range(7):
        for l in range(N_LABELS):
            if (int(T[l]) >> j) & 1:
                M[j] |= np.uint32(1 << l)
    masks = M.view(np.int32)
    ckey = masks.tobytes()
    if _CACHE.get("key") != ckey:
        _CACHE["nc"] = _build_program(masks)
        _CACHE["key"] = ckey
    nc = _CACHE["nc"]

    res = run_bass_kernel_spmd(nc, in_maps, core_ids=list(range(8)))

    img = np.empty((B, C, D, H, W), np.float32)
    labels_out = np.empty((B, C, D, H, W), np.int32)
    jj = np.arange(NGC * GF)
    g_p = (jj % GF) % 16          # partition-in-group of each stream element
    g_i = (jj // GF) * SC + ((jj % GF) // 16)  # pair column index
    for c in range(8):
        b, half = c // 2, c % 2
        h0 = half * HC
        r = res.results[c]
        img[b, 0, :, h0:h0 + HC, :] = (
            np.asarray(r["img"]).astype(np.float32).reshape(D, HC, W))

        lab_flat = np.empty((128, FLAB), np.int32)
        lab_flat[:, :CD] = r["labo"]
        gout = r["gout"]  # [8, NGC*GF] packed pairs
        for grp in range(8):
            v = gout[grp]
            rows = 16 * grp + g_p
            lab_flat[rows, CD + g_i] = v & 127
            lab_flat[rows, CD + NPAIR + g_i] = v >> 7
        labels_out[b, 0, :, h0:h0 + HC, :] = lab_flat.reshape(D, HC, W)
    return img, labels_out


# revision 44
# speedup vs baseline: 1.0342x; 1.0342x over previous
"""Trainium2 Bass kernel for nn_BrainGeneratorModel (bias-field corrupt + per-sample
separable Gaussian blur + label LUT remap), 8-core data/spatial parallel.

Sharding: 8 cores = (sample b in 0..3) x (H-half in 0..1). Each core processes a
[D=192, H=96(+12 halo), W=192] subvolume of one sample plus its label slice.

Per-core pipeline:
  Image (PE/ACT/DVE, zero transposes via stationary-swap matmuls):
    A) stream d-batches: bias matmul (K=4) -> exp (ACT) -> x*expb (DVE tt, bf16)
       -> h-blur with xb as stationary (out partitions = w) -> zq [w,(h',d)] SBUF
    B) stream h'-batches: w-blur with zq-slices as stationary (out partitions = d)
       -> zdd [d,(hl,w')] -> d-blur (normal form) -> PSUM -> img DRAM (f32, direct)
  Labels (DVE and POOL in parallel):
    DVE half: 7 bitplane extractions (Mj >> label) & 1 (reverse0 tensor_scalar)
       + Horner accumulate -> dest value directly.
    POOL half: pairs c = l0 + 32*l1 (host-built) -> gpsimd ap_gather against a
       1024-entry packed LUT (T[l0] + 128*T[l1], int32) -> host unpacks.
"""

import sys

for _p in ("/opt/trn_rl_repo",):
    if _p not in sys.path:
        sys.path.insert(0, _p)

import numpy as np
import ml_dtypes

import concourse.bass as bass
import concourse.mybir as mybir
import concourse.bacc as bacc
import concourse.tile as tile
from concourse.bass_utils import run_bass_kernel_spmd

F32 = mybir.dt.float32
BF16 = mybir.dt.bfloat16
I16 = mybir.dt.int16
I32 = mybir.dt.int32

B, C, D, H, W = 4, 1, 192, 192, 192
SMALL = 4
BIAS_STD = 0.7
MAX_SIGMA = 3.0
TRUNCATE = 4.0
K = 2 * int(TRUNCATE * MAX_SIGMA) + 1  # 25
P = K // 2  # 12
N_LABELS = 32
TABLE = 128

HC = 96            # interior H rows per core
HS = 120           # slab rows = HC + 2*P
DB = 8             # d-batch size (stage A)
NB_A = D // DB     # 24
HB = 8             # h'-batch size (stage B)
NB_B = HC // HB    # 12

FLAB = D * HC * W // 128       # 27648 label cols per partition
CD = 12096                     # DVE-share cols (12 chunks)
CP = FLAB - CD                 # pool share
NPAIR = CP // 2                # 6912 pairs per partition
LCH = 1008                     # DVE label chunk cols
NLC = CD // LCH                # 12 chunks
SC = 144                       # pool idx cols per ap_gather call
NGC = NPAIR // SC              # 54 calls
GF = 16 * SC                   # 2304 gather out free size

_CACHE = {}


def _lin_weights(n_in, n_out):
    pos = np.linspace(0.0, n_in - 1.0, n_out, dtype=np.float64)
    i0 = np.clip(np.floor(pos).astype(np.int64), 0, n_in - 2)
    f = pos - i0
    Wm = np.zeros((n_out, n_in), np.float64)
    r = np.arange(n_out)
    np.add.at(Wm, (r, i0), 1.0 - f)
    np.add.at(Wm, (r, i0 + 1), f)
    return Wm


def _gauss_kernels(sigma3):
    ar = np.arange(K, dtype=np.float64) - K // 2
    out = np.zeros((3, K), np.float64)
    for i, sg in enumerate(sigma3):
        s = max(float(sg), 1e-3)
        g = np.exp(-0.5 * ar * ar / (s * s))
        g = g / g.sum()
        if float(sg) >= 0.01:
            out[i] = g
        else:
            out[i, K // 2] = 1.0
    return out


def _edge_folded_toeplitz(g, n):
    """[n, n] M with out[dst] = sum_src M[src, dst] * x[src], replicate pad."""
    M = np.zeros((n, n), np.float64)
    for j in range(n):
        for t in range(K):
            src = min(max(j + t - P, 0), n - 1)
            M[src, j] += g[t]
    return M


def _slab_toeplitz(g):
    """[HS, HC]: slab rows (pre-clipped by host) -> interior outputs."""
    M = np.zeros((HS, HC), np.float64)
    for j in range(HC):
        for t in range(K):
            M[j + t, j] += g[t]
    return M


def _build_program(masks):
    """masks: 7 int32 bitplane masks of the dest LUT (baked immediates)."""
    nc = bacc.Bacc("TRN2", target_bir_lowering=False, debug=False)

    # ---- external inputs (per core) ----
    xs_h = nc.dram_tensor("xs", [HS, D * W], BF16, kind="ExternalInput")
    c_h = nc.dram_tensor("cydw", [4, D * W], BF16, kind="ExternalInput")
    wht_h = nc.dram_tensor("wht", [4, HS], BF16, kind="ExternalInput")
    gh_h = nc.dram_tensor("gh", [HS, HC], BF16, kind="ExternalInput")
    gw_h = nc.dram_tensor("gw", [W, W], BF16, kind="ExternalInput")
    gd_h = nc.dram_tensor("gd", [D, D], BF16, kind="ExternalInput")
    labd_h = nc.dram_tensor("labd", [128, CD], I32, kind="ExternalInput")
    cpair_h = nc.dram_tensor("cpair", [128, NPAIR], I16, kind="ExternalInput")
    tab2_h = nc.dram_tensor("tab2", [128, 1024], I32, kind="ExternalInput")

    # ---- external outputs ----
    img_h = nc.dram_tensor("img", [D, HC, W], BF16, kind="ExternalOutput")
    labo_h = nc.dram_tensor("labo", [128, CD], I32, kind="ExternalOutput")
    gout_h = nc.dram_tensor("gout", [8, NGC * GF], I32, kind="ExternalOutput")

    with tile.TileContext(nc) as tc:
        with (
            tc.tile_pool(name="consts", bufs=1) as cst,
            tc.tile_pool(name="sxp", bufs=3) as sxp,
            tc.tile_pool(name="cbp", bufs=3) as cbp,
            tc.tile_pool(name="ebp", bufs=3) as ebp,
            tc.tile_pool(name="xbp", bufs=2) as xbp,
            tc.tile_pool(name="zqp", bufs=1) as zqp,
            tc.tile_pool(name="zdp", bufs=2) as zdp,
            tc.tile_pool(name="zip", bufs=2) as zip_,
            tc.tile_pool(name="l32p", bufs=3) as l32p,
            tc.tile_pool(name="ltp", bufs=2) as ltp,
            tc.tile_pool(name="tp", bufs=2) as tp,
            tc.tile_pool(name="accp", bufs=2) as accp,
            tc.tile_pool(name="gp", bufs=2) as gp,  # lag covered by 2 bufs
            tc.tile_pool(name="psA", bufs=3, space="PSUM") as pspA,
            tc.tile_pool(name="ps", bufs=5, space="PSUM") as psp,
        ):
            # ---- constants / persistent tiles ----
            ght = cst.tile([HS, HC], BF16)
            nc.sync.dma_start(ght[:], gh_h.ap())
            gwa = cst.tile([128, W], BF16)
            nc.sync.dma_start(gwa[:], gw_h.ap()[0:128, :])
            gwb = cst.tile([64, W], BF16)
            nc.sync.dma_start(gwb[:], gw_h.ap()[128:192, :])
            gda = cst.tile([128, D], BF16)
            nc.sync.dma_start(gda[:], gd_h.ap()[0:128, :])
            gdb = cst.tile([64, D], BF16)
            nc.sync.dma_start(gdb[:], gd_h.ap()[128:192, :])
            whtt = cst.tile([4, HS], BF16)
            nc.sync.dma_start(whtt[:], wht_h.ap())

            tab2 = cst.tile([128, 1024], I32)
            cpair = cst.tile([128, NPAIR], I16)
            lab_consts_loaded = [False]

            def load_label_consts():
                if not lab_consts_loaded[0]:
                    lab_consts_loaded[0] = True
                    nc.sync.dma_start(tab2[:], tab2_h.ap())
                    nc.sync.dma_start(cpair[:], cpair_h.ap())

            # zq: persistent h/w-blurred volume, [w, (h', d)] bf16
            zqa = zqp.tile([128, HC * D], BF16, tag="zqa")
            zqb = zqp.tile([64, HC * D], BF16, tag="zqb")

            # ================= labels: thunk generators =================
            # DVE bitplane half + POOL ap_gather half, drained into the image
            # loops so every engine queue interleaves label and image work.
            eng = nc.vector

            def _chunk_load(ch):
                sl = slice(ch * LCH, (ch + 1) * LCH)
                l32 = l32p.tile([128, LCH], I32, tag="l32")
                return l32, (lambda: nc.sync.dma_start(
                    l32[:], labd_h.ap()[:, sl]))

            def _one_chunk(ch, l32):
                """Yield compute thunks for one chunk's chain."""
                sl = slice(ch * LCH, (ch + 1) * LCH)
                acc = accp.tile([128, LCH], I32, tag="acc")
                t = tp.tile([128, LCH], I32, tag="t")
                for j in range(6, -1, -1):
                    dst = acc if j == 6 else t

                    def ext(l32=l32, dst=dst, j=j):
                        eng.add_instruction(
                            mybir.InstTensorScalarPtr(
                                name=f"I-{nc.next_id()}",
                                op0=mybir.AluOpType.logical_shift_right,
                                reverse0=True,
                                op1=mybir.AluOpType.bitwise_and,
                                ins=[
                                    eng.lower_ap(l32[:]),
                                    eng.lower_ap_or_imm(int(masks[j]),
                                                        imm_dtype=I32),
                                    eng.lower_ap_or_imm(1, imm_dtype=I32),
                                ],
                                outs=[eng.lower_ap(dst[:])],
                            )
                        )
                    yield ext
                    if j != 6:
                        yield lambda acc=acc, t=t: \
                            nc.vector.scalar_tensor_tensor(
                                acc[:], acc[:], 2, t[:],
                                mybir.AluOpType.mult, mybir.AluOpType.add)
                yield ("defer", lambda acc=acc, sl=sl: nc.sync.dma_start(
                    labo_h.ap()[:, sl], acc[:]))

            def dve_label_thunks():
                # two chunk chains in lockstep to hide per-op latency; loads
                # prefetched one pair ahead; output DMAs deferred one step so
                # they never head-block the SP queue.
                loads = [_chunk_load(ch) for ch in range(2)]
                for th in (loads[0][1], loads[1][1]):
                    th()
                deferred = []
                for cp2 in range((NLC + 1) // 2):
                    # prefetch next pair's loads now (tiles: bufs >= 3)
                    if 2 * cp2 + 2 < NLC:
                        l32n, thn = _chunk_load(2 * cp2 + 2)
                        thn()
                        loads.append((l32n, None))
                    if 2 * cp2 + 3 < NLC:
                        l32n, thn = _chunk_load(2 * cp2 + 3)
                        loads.append((l32n, thn))
                    ga = _one_chunk(2 * cp2, loads[2 * cp2][0])
                    gb = (_one_chunk(2 * cp2 + 1, loads[2 * cp2 + 1][0])
                          if 2 * cp2 + 1 < NLC else iter(()))
                    half_done = False
                    while True:
                        a = next(ga, None)
                        b = next(gb, None)
                        if a is None and b is None:
                            break
                        if not half_done and deferred:
                            # flush prior pair's output DMAs mid-pair
                            for d in deferred:
                                d()
                            deferred = []
                            half_done = True
                        for item in (a, b):
                            if item is None:
                                continue
                            if isinstance(item, tuple) and item[0] == "defer":
                                deferred.append(item[1])
                            else:
                                yield item
                    # late-emitted load of the pair after next
                    if 2 * cp2 + 3 < NLC and loads[2 * cp2 + 3][1] is not None:
                        loads[2 * cp2 + 3][1]()
                        loads[2 * cp2 + 3] = (loads[2 * cp2 + 3][0], None)
                for d in deferred:
                    d()

            def pool_label_thunks():
                pending = []

                def flush():
                    while pending:
                        pending.pop(0)()

                for cp2 in range(NGC // 2):
                    def call(cp2=cp2):
                        g = gp.tile([128, 2 * GF], I32, tag="g")
                        for k in range(2):
                            ca = 2 * cp2 + k
                            nc.gpsimd.ap_gather(
                                g[:, k * GF:(k + 1) * GF], tab2[:],
                                cpair[:, ca * SC:(ca + 1) * SC],
                                128, 1024, 1, GF)
                        flush()
                        pending.append(lambda g=g, cp2=cp2: nc.sync.dma_start(
                            gout_h.ap()[:, 2 * cp2 * GF:(2 * cp2 + 2) * GF],
                            g[0:128:16, :],
                        ))
                    yield call
                yield flush

            dve_it = dve_label_thunks()
            pool_it = pool_label_thunks()
            n_dve_thunks = NLC * 15
            dve_drained = 0.0
            pool_drained = 0.0
            # cumulative pacing weights: stage-A slots lighter (DVE busy with
            # xb mults there), stage-B slots heavier
            WA, WB = 1.0, 5.6
            wtot = NB_A * WA + NB_B * WB

            def drain(slot):
                nonlocal dve_drained, pool_drained
                if slot >= 1:
                    load_label_consts()
                w = ((slot + 1) * WA if slot < NB_A
                     else NB_A * WA + (slot + 1 - NB_A) * WB)
                dve_target = n_dve_thunks * w / wtot
                while dve_drained < dve_target:
                    th = next(dve_it, None)
                    if th is None:
                        break
                    th()
                    dve_drained += 1
                pool_target = (NGC // 2 + 1) * (slot + 1) / 39.0
                while pool_drained < pool_target:
                    th = next(pool_it, None)
                    if th is None:
                        break
                    th()
                    pool_drained += 1

            # ================= image stage A =================
            # xb = x * exp(bias); h-blur via stationary-swap -> zq [w, (h', d)]
            for ib in range(NB_A):
                    d0 = ib * DB
                    sx = sxp.tile([HS, DB * W], BF16)
                    nc.sync.dma_start(sx[:], xs_h.ap()[:, ib * DB * W:
                                                       (ib + 1) * DB * W])
                    cb = cbp.tile([4, DB * W], BF16)
                    nc.sync.dma_start(cb[:], c_h.ap()[:, ib * DB * W:
                                                      (ib + 1) * DB * W])
                    xb = xbp.tile([HS, DB * W], BF16)
                    for q in range(3):
                        qsl = slice(q * 512, (q + 1) * 512)
                        psb = pspA.tile([HS, 512], F32, tag="psb")
                        nc.tensor.matmul(psb[:], whtt[:], cb[:, qsl],
                                         start=True, stop=True)
                        eb = ebp.tile([HS, 512], BF16, tag="eb")
                        nc.scalar.activation(eb[:], psb[:],
                                             mybir.ActivationFunctionType.Exp)
                        nc.vector.tensor_tensor(xb[:, qsl], sx[:, qsl], eb[:],
                                                mybir.AluOpType.mult)
                    # h-blur: stationary = xb slice [h, w-block], rhs = ght
                    # psum [w-block, (dsub 4, h' 96)]
                    for g4 in range(2):
                        pha = psp.tile([128, 4 * HC], F32, tag="ps")
                        phb = psp.tile([64, 4 * HC], F32, tag="ps")
                        for t4 in range(4):
                            dl = g4 * 4 + t4
                            csl = slice(t4 * HC, (t4 + 1) * HC)
                            nc.tensor.matmul(
                                pha[:, csl],
                                xb[:, dl * W: dl * W + 128], ght[:],
                                start=True, stop=True)
                            nc.tensor.matmul(
                                phb[:, csl],
                                xb[:, dl * W + 128: dl * W + 192], ght[:],
                                start=True, stop=True)
                        dq = d0 + 4 * g4
                        nc.scalar.copy(
                            zqa[:].rearrange("p (h d) -> p d h", d=D)
                               [:, dq:dq + 4, :],
                            pha[:].rearrange("p (t h) -> p t h", t=4),
                        )
                        nc.scalar.copy(
                            zqb[:].rearrange("p (h d) -> p d h", d=D)
                               [:, dq:dq + 4, :],
                            phb[:].rearrange("p (t h) -> p t h", t=4),
                        )
                    drain(ib)

            # ================= image stage B =================
            # w-blur: stationary = zq slice [w, d-block] per h',
            #         rhs = gw [w, w'] -> psum [d-block, w'] -> zdd
            # d-blur: lhsT = gd [d, d'-block], rhs = zdd -> psum -> img DRAM
            img_pending = []
            for jb in range(NB_B):
                h0 = jb * HB
                zda = zdp.tile([128, HB * W], BF16, tag="zda")
                zdb = zdp.tile([64, HB * W], BF16, tag="zdb")
                for hp in range(HB // 2):
                    pwa = psp.tile([128, 2 * W], F32, tag="ps")
                    pwb = psp.tile([64, 2 * W], F32, tag="ps")
                    for t2 in range(2):
                        hl = hp * 2 + t2
                        zoff = (h0 + hl) * D
                        csl = slice(t2 * W, (t2 + 1) * W)
                        nc.tensor.matmul(pwa[:, csl],
                                         zqa[:, zoff: zoff + 128], gwa[:],
                                         start=True, stop=False)
                        nc.tensor.matmul(pwa[:, csl],
                                         zqb[:, zoff: zoff + 128], gwb[:],
                                         start=False, stop=True)
                        nc.tensor.matmul(pwb[:, csl],
                                         zqa[:, zoff + 128: zoff + 192], gwa[:],
                                         start=True, stop=False)
                        nc.tensor.matmul(pwb[:, csl],
                                         zqb[:, zoff + 128: zoff + 192], gwb[:],
                                         start=False, stop=True)
                    nc.scalar.copy(
                        zda[:, hp * 2 * W:(hp + 1) * 2 * W], pwa[:])
                    nc.scalar.copy(
                        zdb[:, hp * 2 * W:(hp + 1) * 2 * W], pwb[:])

                # flush previous group's img DMAs (now certainly ready)
                for th in img_pending:
                    th()
                img_pending.clear()
                zia = zip_.tile([128, HB * W], BF16, tag="zia")
                zib = zip_.tile([64, HB * W], BF16, tag="zib")
                for q in range(HB * W // 512):
                    qsl = slice(q * 512, (q + 1) * 512)
                    pia = psp.tile([128, 512], F32, tag="ps")
                    pib = psp.tile([64, 512], F32, tag="ps")
                    nc.tensor.matmul(pia[:], gda[:, 0:128], zda[:, qsl],
                                     start=True, stop=False)
                    nc.tensor.matmul(pia[:], gdb[:, 0:128], zdb[:, qsl],
                                     start=False, stop=True)
                    nc.tensor.matmul(pib[:], gda[:, 128:192], zda[:, qsl],
                                     start=True, stop=False)
                    nc.tensor.matmul(pib[:], gdb[:, 128:192], zdb[:, qsl],
                                     start=False, stop=True)
                    nc.scalar.copy(zia[:, qsl], pia[:])
                    nc.scalar.copy(zib[:, qsl], pib[:])
                # img [d', h, w]: rows h0..h0+8 contiguous per d'
                img_pending.append(lambda zia=zia, h0=h0: nc.sync.dma_start(
                    bass.AP(img_h, h0 * W, [[HC * W, 128], [1, HB * W]]),
                    zia[:],
                ))
                img_pending.append(lambda zib=zib, h0=h0: nc.sync.dma_start(
                    bass.AP(img_h, 128 * HC * W + h0 * W,
                            [[HC * W, 64], [1, HB * W]]),
                    zib[:],
                ))
                drain(NB_A + jb)

            for th in img_pending:
                th()
            img_pending.clear()
            for th in dve_it:
                th()
            for th in pool_it:
                th()
    nc.compile()
    return nc


def _host_prep(x, small_bias, sigma01, labels, source_values, dest_values):
    Wd = _lin_weights(SMALL, D)
    Whm = _lin_weights(SMALL, H)
    Wwm = _lin_weights(SMALL, W)

    mapping = np.zeros(TABLE, np.int64)
    mapping[np.asarray(source_values, np.int64)] = np.asarray(dest_values, np.int64)
    T = mapping[:N_LABELS]

    c_all = np.arange(1024)
    tab2 = (T[c_all % 32] + 128 * T[c_all // 32]).astype(np.int32)
    tab2_rep = np.broadcast_to(tab2, (128, 1024)).copy()

    in_maps = []
    for c in range(8):
        b, half = c // 2, c % 2
        h0 = half * HC
        hidx = np.clip(np.arange(h0 - P, h0 + HC + P), 0, H - 1)

        # x slab, h-major bf16: [HS, D, W]
        xs = np.asarray(x[b, 0], np.float32)[:, hidx, :].transpose(1, 0, 2)
        xs = np.ascontiguousarray(xs, dtype=ml_dtypes.bfloat16).reshape(HS, D * W)

        sm = np.asarray(small_bias[b, 0], np.float64) * BIAS_STD
        Cydw = np.einsum("xyz,dx,wz->ydw", sm, Wd, Wwm).reshape(4, D * W)
        WhT = np.ascontiguousarray(Whm[hidx, :].T)

        g3 = _gauss_kernels(np.asarray(sigma01[b], np.float64) * MAX_SIGMA)
        Gh = _slab_toeplitz(g3[1])
        Gw = _edge_folded_toeplitz(g3[2], W)
        Gd = _edge_folded_toeplitz(g3[0], D)

        lab = np.asarray(labels[b, 0][:, h0:h0 + HC, :], np.int32).reshape(128, FLAB)
        labd = np.ascontiguousarray(lab[:, :CD])
        lp = lab[:, CD:]
        cpair = (lp[:, NPAIR:] * 32 + lp[:, :NPAIR]).astype(np.int16)

        in_maps.append({
            "xs": xs,
            "cydw": Cydw.astype(ml_dtypes.bfloat16),
            "wht": WhT.astype(ml_dtypes.bfloat16),
            "gh": Gh.astype(ml_dtypes.bfloat16),
            "gw": Gw.astype(ml_dtypes.bfloat16),
            "gd": Gd.astype(ml_dtypes.bfloat16),
            "labd": labd,
            "cpair": np.ascontiguousarray(cpair),
            "tab2": tab2_rep,
        })
    return in_maps, T


def kernel(x, small_bias, sigma01, labels, source_values, dest_values):
    in_maps, T = _host_prep(x, small_bias, sigma01, labels,
                            source_values, dest_values)

    # bitplane masks of the LUT (program immediates -> cache key)
    M = np.zeros(7, dtype=np.uint32)
    for j in range(7):
        for l in range(N_LABELS):
            if (int(T[l]) >> j) & 1:
                M[j] |= np.uint32(1 << l)
    masks = M.view(np.int32)
    ckey = masks.tobytes()
    if _CACHE.get("key") != ckey:
        _CACHE["nc"] = _build_program(masks)
        _CACHE["key"] = ckey
    nc = _CACHE["nc"]

    res = run_bass_kernel_spmd(nc, in_maps, core_ids=list(range(8)))

    img = np.empty((B, C, D, H, W), np.float32)
    labels_out = np.empty((B, C, D, H, W), np.int32)
    jj = np.arange(NGC * GF)
    g_p = (jj % GF) % 16          # partition-in-group of each stream element
    g_i = (jj // GF) * SC + ((jj % GF) // 16)  # pair column index
    for c in range(8):
        b, half = c // 2, c % 2
        h0 = half * HC
        r = res.results[c]
        img[b, 0, :, h0:h0 + HC, :] = (
            np.asarray(r["img"]).astype(np.float32).reshape(D, HC, W))

        lab_flat = np.empty((128, FLAB), np.int32)
        lab_flat[:, :CD] = r["labo"]
        gout = r["gout"]  # [8, NGC*GF] packed pairs
        for grp in range(8):
            v = gout[grp]
            rows = 16 * grp + g_p
            lab_flat[rows, CD + g_i] = v & 127
            lab_flat[rows, CD + NPAIR + g_i] = v >> 7
        labels_out[b, 0, :, h0:h0 + HC, :] = lab_flat.reshape(D, HC, W)
    return img, labels_out
